# revision 10
# baseline (speedup 1.0000x reference)
"""Trainium2 Bass kernel for 3x3 valid Conv2D (NCHW, OIHW), batch-parallel on 8 cores.

x(32,64,130,130) conv w(128,64,3,3) -> (32,128,128,128), plus bias(128,)
broadcast against the LAST axis (Wo) of the output (faithful to the
reference's torch-style broadcast, which requires Wo == K == 128).

Strategy per core (4 images):
  - x stored in SBUF with row-parity interleave: partitions 0-63 = channels
    (even image rows), partitions 64-127 = channels (odd image rows). A tap
    pair (u, u+1) then reads both halves at ONE free-dim offset, so two
    64-deep taps fuse into one 128-deep matmul -- no data duplication.
  - Per 4 same-parity output rows (one PSUM tile [128k, 512px]):
    3 fused pair-matmuls (128-deep) + 3 single-tap matmuls (64-deep).
    The even-tile single (rows 0-63) and odd-tile single (rows 64-127) are
    issued back-to-back into distinct PE row-groups so they run concurrently.
    => 9 matmul slots per 8 output rows = 100% PE array utilization.
  - float32r matmuls (~1.3e-4 rel err vs fp32 reference).
  - x is DMA'd in 4 chunks per image (17 row-pairs each, 1-row overlap) so
    compute starts ~3us in instead of waiting for a whole image.
  - DVE evicts PSUM -> row-interleaved SBUF staging [128, 2048] (16 output
    rows); one 1-MB DMA out per 16-row block writes 8KB-contiguous chunks
    per output channel. Output DMAs ride the Scalar engine's HW queue so
    they overlap input loads on the Sync queue.
"""
import numpy as np

B, C, K, H, W = 32, 64, 128, 130, 130
HO = WO = 128
NCORES = 8
BLOC = B // NCORES  # 4 images per core
T = 65              # parity row-pairs (rows 0..129 -> 65 even + 65 odd)
NG = 16             # groups of 8 output rows per image
TC = 9              # row-pairs per x chunk (2 groups + 1 overlap row)
NCHUNK = 8
COMPUTE = "f32r"   # "f32r" | "bf16"

_CACHE = {}


def _build(with_bias: bool, compute: str = "f32r"):
    import concourse.tile as tile
    from concourse import bacc, mybir

    nc = bacc.Bacc("TRN2", target_bir_lowering=False, debug=False)
    f32 = mybir.dt.float32
    cdt = mybir.dt.float32r if compute == "f32r" else mybir.dt.bfloat16

    x_d = nc.dram_tensor("xloc", [BLOC, 128, T * W], cdt, kind="ExternalInput")
    w_d = nc.dram_tensor("wpk", [128, 1152], cdt, kind="ExternalInput")
    o_d = nc.dram_tensor("out", [BLOC, K, HO, WO], f32, kind="ExternalOutput")
    if with_bias:
        b_d = nc.dram_tensor("btile", [128, 512], f32, kind="ExternalInput")

    o_flat = o_d.ap().rearrange("b k i j -> b k (i j)")
    x_flat = x_d.ap().rearrange("b p (t j) -> b p t j", j=W)

    with tile.TileContext(nc) as tc:
        with (
            tc.tile_pool(name="wpool", bufs=1) as wpool,
            tc.tile_pool(name="xpool", bufs=10) as xpool,
            tc.tile_pool(name="spool", bufs=3) as spool,
            tc.tile_pool(name="psum", bufs=3, space="PSUM") as psum,
        ):
            # weights ride the (initially idle) scalar queue so they don't
            # delay the first x chunk on the sync queue
            wt = wpool.tile([128, 1152], cdt)
            nc.scalar.dma_start(wt[:], w_d.ap()[:, :])
            if with_bias:
                bt = wpool.tile([128, 512], f32, tag="bias")
                nc.scalar.dma_start(bt[:], b_d.ap()[:, :])

            for b in range(BLOC):
                xvs = []
                for c in range(NCHUNK):
                    xt = xpool.tile([128, TC * W], cdt)
                    nc.sync.dma_start(xt[:], x_flat[b, :, 8 * c:8 * c + TC, :])
                    xvs.append(xt[:].rearrange("p (t j) -> p t j", j=W))

                for hblk in range(NG // 2):
                    st = spool.tile([128, 2048], f32)
                    sv = st[:].rearrange("p (r j) -> p r j", j=WO)
                    for sub in range(2):
                        g = 2 * hblk + sub
                        xv = xvs[g // 2]
                        lm = 4 * (g % 2)  # m0 local to chunk
                        pe = psum.tile([128, 512], f32, tag="pe")
                        po = psum.tile([128, 512], f32, tag="po")
                        # fused tap-pairs: even (u0,u1)@t=m, odd (u1,u2)@t=m+1
                        for vi in range(3):
                            nc.tensor.matmul(
                                pe[:], wt[:, 128 * vi:128 * (vi + 1)],
                                xv[:, lm:lm + 4, vi:vi + 128],
                                start=(vi == 0), stop=False,
                            )
                            nc.tensor.matmul(
                                po[:], wt[:, 384 + 128 * vi:384 + 128 * (vi + 1)],
                                xv[:, lm + 1:lm + 5, vi:vi + 128],
                                start=(vi == 0), stop=False,
                            )
                        # leftover singles -> distinct PE row groups, concurrent
                        for vi in range(3):
                            nc.tensor.matmul(
                                pe[:], wt[0:64, 768 + 128 * vi:768 + 128 * (vi + 1)],
                                xv[0:64, lm + 1:lm + 5, vi:vi + 128],
                                start=False, stop=(vi == 2),
                            )
                            nc.tensor.matmul(
                                po[:], wt[64:128, 768 + 128 * vi:768 + 128 * (vi + 1)],
                                xv[64:128, lm:lm + 4, vi:vi + 128],
                                start=False, stop=(vi == 2),
                            )
                        if with_bias:
                            nc.vector.tensor_add(sv[:, 8 * sub:8 * sub + 8:2, :], pe[:], bt[:])
                            nc.vector.tensor_add(sv[:, 8 * sub + 1:8 * sub + 8:2, :], po[:], bt[:])
                        else:
                            nc.vector.tensor_copy(sv[:, 8 * sub:8 * sub + 8:2, :], pe[:])
                            nc.vector.tensor_copy(sv[:, 8 * sub + 1:8 * sub + 8:2, :], po[:])
                    nc.scalar.dma_start(
                        o_flat[b, :, (16 * hblk) * WO:(16 * hblk + 16) * WO], st[:]
                    )
    nc.compile()
    return nc


def _get_nc(with_bias: bool, compute: str = COMPUTE):
    key = ("conv", with_bias, compute)
    if key not in _CACHE:
        _CACHE[key] = _build(with_bias, compute)
    return _CACHE[key]


def _prep_inputs(x, weight, bias, with_bias, compute: str = COMPUTE):
    xs = x.reshape(NCORES, BLOC, C, H, W)
    xr = np.empty((NCORES, BLOC, 128, T * W), np.float32)
    xr[:, :, 0:64] = xs[:, :, :, 0::2, :].reshape(NCORES, BLOC, C, T * W)
    xr[:, :, 64:128] = xs[:, :, :, 1::2, :].reshape(NCORES, BLOC, C, T * W)

    wkc = np.ascontiguousarray(weight.transpose(2, 3, 1, 0))  # [u, v, c, k]
    wpk = np.empty((128, 1152), np.float32)
    for v in range(3):
        wpk[0:64, 128 * v:128 * (v + 1)] = wkc[0, v]        # even pair lower: u0
        wpk[64:128, 128 * v:128 * (v + 1)] = wkc[1, v]      # even pair upper: u1
        wpk[0:64, 384 + 128 * v:384 + 128 * (v + 1)] = wkc[1, v]    # odd pair lower: u1
        wpk[64:128, 384 + 128 * v:384 + 128 * (v + 1)] = wkc[2, v]  # odd pair upper: u2
        wpk[0:64, 768 + 128 * v:768 + 128 * (v + 1)] = wkc[2, v]    # even single: u2
        wpk[64:128, 768 + 128 * v:768 + 128 * (v + 1)] = wkc[0, v]  # odd single: u0

    if compute == "bf16":
        import ml_dtypes
        xr = xr.astype(ml_dtypes.bfloat16)
        wpk = wpk.astype(ml_dtypes.bfloat16)
    in_maps = []
    for core in range(NCORES):
        m = {"xloc": xr[core], "wpk": wpk}
        if with_bias:
            m["btile"] = np.tile(bias, (128, 4))  # bias[j] along free dim
        in_maps.append(m)
    return in_maps


def kernel(x, weight, bias):
    from concourse.bass_utils import run_bass_kernel_spmd

    x = np.ascontiguousarray(np.asarray(x, dtype=np.float32))
    weight = np.asarray(weight, dtype=np.float32)
    bias = np.asarray(bias, dtype=np.float32)
    with_bias = bool(np.any(bias))

    nc = _get_nc(with_bias)
    in_maps = _prep_inputs(x, weight, bias, with_bias)
    res = run_bass_kernel_spmd(nc, in_maps, core_ids=list(range(NCORES)))
    out = np.empty((B, K, HO, WO), np.float32)
    for core in range(NCORES):
        out[core * BLOC:(core + 1) * BLOC] = res.results[core]["out"]
    return out


# revision 12
# speedup vs baseline: 1.1885x; 1.1885x over previous
"""Trainium2 Bass kernel for 3x3 valid Conv2D (NCHW, OIHW), batch-parallel on 8 cores.

x(32,64,130,130) conv w(128,64,3,3) -> (32,128,128,128), plus bias(128,)
broadcast against the LAST axis (Wo) of the output (faithful to the
reference's torch-style broadcast, which requires Wo == K == 128).

Strategy per core (4 images):
  - x stored in SBUF with row-parity interleave: partitions 0-63 = channels
    (even image rows), partitions 64-127 = channels (odd image rows). A tap
    pair (u, u+1) then reads both halves at ONE free-dim offset, so two
    64-deep taps fuse into one 128-deep matmul -- no data duplication.
  - Per 4 same-parity output rows (one PSUM tile [128k, 512px]):
    3 fused pair-matmuls (128-deep) + 3 single-tap matmuls (64-deep).
    The even-tile single (rows 0-63) and odd-tile single (rows 64-127) are
    issued back-to-back into distinct PE row-groups so they run concurrently.
    => 9 matmul slots per 8 output rows = 100% PE array utilization.
  - float32r matmuls (~1.3e-4 rel err vs fp32 reference).
  - x is DMA'd in 4 chunks per image (17 row-pairs each, 1-row overlap) so
    compute starts ~3us in instead of waiting for a whole image.
  - DVE evicts PSUM -> row-interleaved SBUF staging [128, 2048] (16 output
    rows); one 1-MB DMA out per 16-row block writes 8KB-contiguous chunks
    per output channel. Output DMAs ride the Scalar engine's HW queue so
    they overlap input loads on the Sync queue.
"""
import numpy as np

B, C, K, H, W = 32, 64, 128, 130, 130
HO = WO = 128
NCORES = 8
BLOC = B // NCORES  # 4 images per core
T = 65              # parity row-pairs (rows 0..129 -> 65 even + 65 odd)
NG = 16             # groups of 8 output rows per image
TC = 9              # row-pairs per x chunk (2 groups + 1 overlap row)
NCHUNK = 8
COMPUTE = "f32r"   # "f32r" | "bf16"

_CACHE = {}


def _build(with_bias: bool, compute: str = "f32r"):
    import concourse.tile as tile
    from concourse import bacc, mybir

    nc = bacc.Bacc("TRN2", target_bir_lowering=False, debug=False)
    f32 = mybir.dt.float32
    cdt = mybir.dt.float32r if compute == "f32r" else mybir.dt.bfloat16

    x_d = nc.dram_tensor("xloc", [BLOC, 128, T * W], cdt, kind="ExternalInput")
    w_d = nc.dram_tensor("wpk", [128, 1152], cdt, kind="ExternalInput")
    o_d = nc.dram_tensor("out", [BLOC, K, HO, WO], f32, kind="ExternalOutput")
    if with_bias:
        b_d = nc.dram_tensor("btile", [128, 512], f32, kind="ExternalInput")

    o_flat = o_d.ap().rearrange("b k i j -> b k (i j)")
    x_flat = x_d.ap().rearrange("b p (t j) -> b p t j", j=W)

    with tile.TileContext(nc) as tc:
        with (
            tc.tile_pool(name="wpool", bufs=1) as wpool,
            tc.tile_pool(name="xpool", bufs=10) as xpool,
            tc.tile_pool(name="spool", bufs=3) as spool,
            tc.tile_pool(name="psum", bufs=3, space="PSUM") as psum,
        ):
            # weights ride the (initially idle) scalar queue so they don't
            # delay the first x chunk on the sync queue
            wt = wpool.tile([128, 1152], cdt)
            nc.scalar.dma_start(wt[:], w_d.ap()[:, :])
            if with_bias:
                bt = wpool.tile([128, 512], f32, tag="bias")
                nc.scalar.dma_start(bt[:], b_d.ap()[:, :])

            for b in range(BLOC):
                xvs = []
                for c in range(NCHUNK):
                    xt = xpool.tile([128, TC * W], cdt)
                    nc.sync.dma_start(xt[:], x_flat[b, :, 8 * c:8 * c + TC, :])
                    xvs.append(xt[:].rearrange("p (t j) -> p t j", j=W))

                for hblk in range(NG // 2):
                    st = spool.tile([128, 2048], f32)
                    sv = st[:].rearrange("p (r j) -> p r j", j=WO)
                    for sub in range(2):
                        g = 2 * hblk + sub
                        xv = xvs[g // 2]
                        lm = 4 * (g % 2)  # m0 local to chunk
                        pe = psum.tile([128, 512], f32, tag="pe")
                        po = psum.tile([128, 512], f32, tag="po")
                        # fused tap-pairs: even (u0,u1)@t=m, odd (u1,u2)@t=m+1
                        for vi in range(3):
                            nc.tensor.matmul(
                                pe[:], wt[:, 128 * vi:128 * (vi + 1)],
                                xv[:, lm:lm + 4, vi:vi + 128],
                                start=(vi == 0), stop=False,
                            )
                            nc.tensor.matmul(
                                po[:], wt[:, 384 + 128 * vi:384 + 128 * (vi + 1)],
                                xv[:, lm + 1:lm + 5, vi:vi + 128],
                                start=(vi == 0), stop=False,
                            )
                        # leftover singles -> distinct PE row groups, concurrent
                        for vi in range(3):
                            nc.tensor.matmul(
                                pe[:], wt[0:64, 768 + 128 * vi:768 + 128 * (vi + 1)],
                                xv[0:64, lm + 1:lm + 5, vi:vi + 128],
                                start=False, stop=(vi == 2),
                            )
                            nc.tensor.matmul(
                                po[:], wt[64:128, 768 + 128 * vi:768 + 128 * (vi + 1)],
                                xv[64:128, lm:lm + 4, vi:vi + 128],
                                start=False, stop=(vi == 2),
                            )
                        if with_bias:
                            nc.vector.tensor_add(sv[:, 8 * sub:8 * sub + 8:2, :], pe[:], bt[:])
                            nc.vector.tensor_add(sv[:, 8 * sub + 1:8 * sub + 8:2, :], po[:], bt[:])
                        else:
                            nc.vector.tensor_copy(sv[:, 8 * sub:8 * sub + 8:2, :], pe[:])
                            nc.vector.tensor_copy(sv[:, 8 * sub + 1:8 * sub + 8:2, :], po[:])
                    nc.scalar.dma_start(
                        o_flat[b, :, (16 * hblk) * WO:(16 * hblk + 16) * WO], st[:]
                    )
    nc.compile()
    return nc


def _get_nc(with_bias: bool, compute: str = None):
    compute = compute or COMPUTE
    key = ("conv", with_bias, compute)
    if key not in _CACHE:
        _CACHE[key] = _build(with_bias, compute)
    return _CACHE[key]


def _prep_inputs(x, weight, bias, with_bias, compute: str = None):
    compute = compute or COMPUTE
    xs = x.reshape(NCORES, BLOC, C, H, W)
    xr = np.empty((NCORES, BLOC, 128, T * W), np.float32)
    xr[:, :, 0:64] = xs[:, :, :, 0::2, :].reshape(NCORES, BLOC, C, T * W)
    xr[:, :, 64:128] = xs[:, :, :, 1::2, :].reshape(NCORES, BLOC, C, T * W)

    wkc = np.ascontiguousarray(weight.transpose(2, 3, 1, 0))  # [u, v, c, k]
    wpk = np.empty((128, 1152), np.float32)
    for v in range(3):
        wpk[0:64, 128 * v:128 * (v + 1)] = wkc[0, v]        # even pair lower: u0
        wpk[64:128, 128 * v:128 * (v + 1)] = wkc[1, v]      # even pair upper: u1
        wpk[0:64, 384 + 128 * v:384 + 128 * (v + 1)] = wkc[1, v]    # odd pair lower: u1
        wpk[64:128, 384 + 128 * v:384 + 128 * (v + 1)] = wkc[2, v]  # odd pair upper: u2
        wpk[0:64, 768 + 128 * v:768 + 128 * (v + 1)] = wkc[2, v]    # even single: u2
        wpk[64:128, 768 + 128 * v:768 + 128 * (v + 1)] = wkc[0, v]  # odd single: u0

    if compute == "bf16":
        import ml_dtypes
        xr = xr.astype(ml_dtypes.bfloat16)
        wpk = wpk.astype(ml_dtypes.bfloat16)
    in_maps = []
    for core in range(NCORES):
        m = {"xloc": xr[core], "wpk": wpk}
        if with_bias:
            m["btile"] = np.tile(bias, (128, 4))  # bias[j] along free dim
        in_maps.append(m)
    return in_maps


def kernel(x, weight, bias):
    from concourse.bass_utils import run_bass_kernel_spmd

    x = np.ascontiguousarray(np.asarray(x, dtype=np.float32))
    weight = np.asarray(weight, dtype=np.float32)
    bias = np.asarray(bias, dtype=np.float32)
    with_bias = bool(np.any(bias))

    nc = _get_nc(with_bias)
    in_maps = _prep_inputs(x, weight, bias, with_bias)
    res = run_bass_kernel_spmd(nc, in_maps, core_ids=list(range(NCORES)))
    out = np.empty((B, K, HO, WO), np.float32)
    for core in range(NCORES):
        out[core * BLOC:(core + 1) * BLOC] = res.results[core]["out"]
    return out
